# revision 64
# baseline (speedup 1.0000x reference)
"""Trainium2 Bass kernel for nn_Attention_20461224198682.

Multi-head attention (B=64, N=196, C=768, H=12, D=64) with relative position
bias and key masking. Data-parallel over batch across 8 NeuronCores (8
batches/core). All device compute in a transposed layout (feature dim on
partitions) so no on-device transposes are needed:

  qkv^T[o,t]  = Wqkv^T-as-lhsT @ x^T    (q columns pre-scaled by D^-0.5;
                                         x^T / W^T / bf16 casts done on host)
  s^T[m,n]    = k^T-as-lhsT @ q^T       (keys m on partitions)
  p[m,n]      = exp(s^T) * exp(bias)^T  (rpe bias exponentiated on host,
                                         applied multiplicatively post-exp)
  o^T[d,n]    = v-as-lhsT @ p           (v rows are pre-scaled by the key
                                         mask 0/1 - this also masks the ones
                                         column, so row 64 of o^T is exactly
                                         the masked softmax denominator)
  o^T[0:64]  /= denom                   (approx reciprocal on DVE + GpSimd
                                         partition_broadcast + DVE multiply)
  out^T[o2,t] = Wproj^T-as-lhsT @ concat_h o^T   (+ proj bias, which also
                                         absorbs the v bias: softmax rows
                                         sum to 1, so +vb is exact)

Matmuls run in bf16 (f32 PSUM accumulation); softmax path in f32. Emission
order streams the first 1.5 batch-pairs of attention behind the q/k
projection chains so PE, ACT, DVE and GpSimd overlap from ~15us on.
"""
import numpy as np
import ml_dtypes

B, N, C, H = 64, 196, 768, 12
D = C // H
SCALE = D ** -0.5
TABLE = 729
NCORES = 8
NB = B // NCORES          # batches per core
TOK = NB * N              # tokens per core
MCH = [(0, 128), (128, 68)]   # key-token chunks within a batch
BF16 = ml_dtypes.bfloat16

_cache = {}


def _build_nc():
    import concourse.bass as bass
    import concourse.tile as tile
    from concourse import bacc, mybir

    f32 = mybir.dt.float32
    bf16 = mybir.dt.bfloat16
    AF = mybir.ActivationFunctionType
    AOT = mybir.AluOpType

    nc = bacc.Bacc()
    xt_d = nc.declare_dram_parameter("xt", [C, TOK], bf16, isOutput=False)
    wqkv_d = nc.declare_dram_parameter("wqkv", [C, 3 * C], bf16, isOutput=False)
    qkb_d = nc.declare_dram_parameter("qkb", [128, 12], f32, isOutput=False)
    wproj_d = nc.declare_dram_parameter("wproj", [C, C], bf16, isOutput=False)
    pjb_d = nc.declare_dram_parameter("pjb", [128, 6], f32, isOutput=False)
    biasT_d = nc.declare_dram_parameter("biasT", [128, H * 2 * N], bf16, isOutput=False)
    maskp_d = nc.declare_dram_parameter("maskp", [128, NB * 2], f32, isOutput=False)
    out_d = nc.declare_dram_parameter("out", [C, TOK], f32, isOutput=True)

    NCH = [(i * 392, 392) for i in range(4)]  # token chunks for qk projection

    with tile.TileContext(nc) as tc:
        from contextlib import ExitStack
        with ExitStack() as ctx:
            p_w = ctx.enter_context(tc.tile_pool(name="w", bufs=1))
            p_xt = ctx.enter_context(tc.tile_pool(name="xt", bufs=6))
            p_qk = ctx.enter_context(tc.tile_pool(name="qk", bufs=48))
            p_vx = ctx.enter_context(tc.tile_pool(name="vx", bufs=16))
            p_small = ctx.enter_context(tc.tile_pool(name="small", bufs=8))
            p_sm = ctx.enter_context(tc.tile_pool(name="sm", bufs=6))
            p_cc = ctx.enter_context(tc.tile_pool(name="cc", bufs=12))
            p_ot = ctx.enter_context(tc.tile_pool(name="ot", bufs=4))
            pp = ctx.enter_context(tc.tile_pool(name="psum", bufs=8, space="PSUM"))

            # ---- persistent inputs (split DMAs so compute starts early) ----
            maskp = p_small.tile([128, NB * 2], f32, tag="maskp")
            nc.sync.dma_start(maskp[:], maskp_d[:])
            qkb = p_small.tile([128, 12], f32, tag="qkb")
            nc.sync.dma_start(qkb[:], qkb_d[:])
            pjb = p_small.tile([128, 6], f32, tag="pjb")
            nc.sync.dma_start(pjb[:], pjb_d[:])
            wq = [p_w.tile([128, 3 * C], bf16, tag="wq", bufs=6, name=f"wq{c}")
                  for c in range(6)]
            xt = [p_xt.tile([128, TOK], bf16, tag="xt", bufs=6, name=f"xt{c}")
                  for c in range(6)]
            for c in range(6):
                nc.sync.dma_start(wq[c][:, 2 * C:3 * C],
                                  wqkv_d[c * 128:(c + 1) * 128, 2 * C:3 * C])
                nc.scalar.dma_start(xt[c][:, 0:392],
                                    xt_d[c * 128:(c + 1) * 128, 0:392])
            for c in range(6):
                nc.sync.dma_start(wq[c][:, 0:C], wqkv_d[c * 128:(c + 1) * 128, 0:C])
                nc.sync.dma_start(xt[c][:, 392:784],
                                  xt_d[c * 128:(c + 1) * 128, 392:784])
            for c in range(6):
                nc.sync.dma_start(wq[c][:, C:2 * C],
                                  wqkv_d[c * 128:(c + 1) * 128, C:2 * C])
                nc.sync.dma_start(xt[c][:, 784:TOK],
                                  xt_d[c * 128:(c + 1) * 128, 784:TOK])
            wp = []
            for c in range(6):
                t = p_w.tile([128, C], bf16, tag="wp", bufs=6)
                nc.sync.dma_start(t[:], wproj_d[c * 128:(c + 1) * 128, :])
                wp.append(t)
            bias_t = p_w.tile([128, H * 2 * N], bf16, tag="biasT")
            nc.sync.dma_start(bias_t[:], biasT_d[:])
            ones12 = p_small.tile([128, 12], bf16, tag="ones12")
            nc.vector.memset(ones12[:], 1.0)

            # ---- v projection (batch-aligned, 65-strided heads + ones col) ----
            # v bias is folded into the proj bias on the host (softmax rows
            # sum to one), so no rank-1 bias matmul here.
            vx = []
            for b in range(NB):
                for mc, msz in MCH:
                    vt = p_vx.tile([128, H * 65], bf16, tag="vx", bufs=16)
                    mci = mc // 128
                    b_ = len(vx) // 2
                    ones_cols = vt[:, :].rearrange("p (h e) -> p h e", e=65)[:, :, 64:65]
                    nc.scalar.activation(
                        ones_cols, ones12[:, :].rearrange("p (h e) -> p h e", e=1),
                        AF.Copy, scale=maskp[:, b_ * 2 + mci:b_ * 2 + mci + 1])
                    for o0, hoff in ((0, 0), (384, 6)):
                        ps = pp.tile([128, 392], f32, tag="ps", bufs=3)
                        for c in range(6):
                            nc.tensor.matmul(
                                ps[:msz, :384],
                                xt[c][:, b * N + mc: b * N + mc + msz],
                                wq[c][:, 2 * C + o0: 2 * C + o0 + 384],
                                start=(c == 0), stop=(c == 5),
                            )
                        dst = vt[:msz, hoff * 65:(hoff + 6) * 65].rearrange(
                            "p (h e) -> p h e", e=65)[:, :, 0:64]
                        src = ps[:msz, :384].rearrange("p (h e) -> p h e", e=64)
                        if hoff == 0:
                            nc.scalar.activation(
                                dst, src, AF.Copy,
                                scale=maskp[:msz, b_ * 2 + mci:b_ * 2 + mci + 1])
                        else:
                            nc.vector.tensor_scalar(
                                dst, src,
                                maskp[:msz, b_ * 2 + mci:b_ * 2 + mci + 1],
                                None, op0=AOT.mult)
                    vx.append(vt)

            # ---- q,k projection: qk[j][ch] [128, 392] = (Wqkv^T)^T x^T ----
            qk = [[None] * 4 for _ in range(12)]
            def qkproj(j):
                for chi, (t0, tw) in enumerate(NCH):
                    ps = pp.tile([128, 392], f32, tag="ps", bufs=3)
                    for c in range(6):
                        nc.tensor.matmul(
                            ps[:, :tw],
                            wq[c][:, j * 128:(j + 1) * 128],
                            xt[c][:, t0:t0 + tw],
                            start=(c == 0), stop=(c == 5),
                        )
                    qt = p_qk.tile([128, 392], bf16, tag="qk", bufs=48)
                    nc.scalar.activation(qt[:, :tw], ps[:, :tw], AF.Identity,
                                         bias=qkb[:, j:j + 1], scale=1.0)
                    qk[j][chi] = qt

            # ---- attention + output projection, per pair of batches ----
            def attention(b, jq, cc):
                bi = b % 2
                tb = (b % 2) * N       # token offset inside the 392-chunk
                chb = b // 2           # which 392-chunk holds batch b
                ps_o = pp.tile([65, 2 * N], f32, tag="pso", bufs=3)
                for hi in range(2):
                    h = 2 * jq + hi
                    po = hi * 64
                    qAP = qk[jq][chb][po:po + 64, tb:tb + N]
                    pt = p_sm.tile([128, 2 * N], bf16, tag="pt", bufs=10)
                    for mci, (mc, msz) in enumerate(MCH):
                        kAP = qk[6 + jq][chb][po:po + 64, tb + mc:tb + mc + msz]
                        ps_s = pp.tile([128, N], f32, tag="ps", bufs=3)
                        nc.tensor.matmul(ps_s[:msz, :], kAP, qAP,
                                         start=True, stop=True)
                        p0 = p_sm.tile([128, N], bf16, tag="p0", bufs=10)
                        nc.scalar.activation(p0[:msz, :], ps_s[:msz, :], AF.Exp)
                        # pt = exp(s) * exp(bias); mask lives in v
                        nc.vector.tensor_mul(
                            pt[:msz, mci * N:(mci + 1) * N], p0[:msz, :],
                            bias_t[:msz, (h * 2 + mci) * N:(h * 2 + mci + 1) * N])
                    for mci, (mc, msz) in enumerate(MCH):
                        vsl = vx[b * 2 + mci][0:msz, h * 65:h * 65 + 65]
                        nc.tensor.matmul(
                            ps_o[:, hi * N:(hi + 1) * N], vsl,
                            pt[:msz, mci * N:(mci + 1) * N],
                            start=(mci == 0), stop=(mci == 1))
                den = p_sm.tile([1, 2 * N], f32, tag="den", bufs=8)
                nc.any.tensor_copy(den[:, :], ps_o[64:65, :])
                rec = p_sm.tile([1, 2 * N], f32, tag="rec", bufs=8)
                nc.vector.reciprocal_approx_fast(rec[:, :], den[:, :])
                rb = p_sm.tile([64, 2 * N], f32, tag="rb", bufs=8)
                nc.gpsimd.partition_broadcast(rb[:, :], rec[:, :])
                for hi in range(2):
                    nc.vector.tensor_mul(
                        cc[jq][hi * 64:hi * 64 + 64, bi * N:(bi + 1) * N],
                        ps_o[0:64, hi * N:(hi + 1) * N],
                        rb[:, hi * N:(hi + 1) * N])

            def proj(bp, cc, bi_list=(0, 1)):
                w = len(bi_list) * N
                c0 = bi_list[0] * N
                for o2 in range(6):
                    ps = pp.tile([128, 392], f32, tag="pj", bufs=2)
                    for c2 in range(6):
                        nc.tensor.matmul(ps[:, :w],
                                         wp[c2][:, o2 * 128:(o2 + 1) * 128],
                                         cc[c2][:, c0:c0 + w],
                                         start=(c2 == 0), stop=(c2 == 5))
                    ot = p_ot.tile([128, 2 * N], f32, tag="ot", bufs=4)
                    nc.scalar.activation(ot[:, :w], ps[:, :w], AF.Identity,
                                         bias=pjb[:, o2:o2 + 1], scale=1.0)
                    nc.sync.dma_start(
                        out_d[o2 * 128:(o2 + 1) * 128,
                              (2 * bp) * N + c0:(2 * bp) * N + c0 + w],
                        ot[:, :w])

            def make_cc():
                cc = []
                for j in range(6):
                    cct = p_cc.tile([128, 2 * N], bf16, tag="cc", bufs=18,
                                    name=f"cct{j}")
                    cc.append(cct)
                return cc

            cc0 = make_cc()
            cc1 = make_cc()
            for jq in range(6):
                qkproj(jq)
                qkproj(6 + jq)
                attention(0, jq, cc0)
                attention(1, jq, cc0)
                attention(2, jq, cc1)
            proj(0, cc0)
            for jq in range(6):
                attention(3, jq, cc1)
            proj(1, cc1)
            for bp in range(2, NB // 2):
                cc = make_cc()
                last = (bp == NB // 2 - 1)
                for bi in range(2):
                    for jq in range(6):
                        attention(2 * bp + bi, jq, cc)
                    if last:
                        proj(bp, cc, bi_list=(bi,))
                if not last:
                    proj(bp, cc)

    nc.finalize()
    return nc


def _prep_in_maps(x, qkv_w, qkv_b, proj_w, proj_b, rpe_table, rpe_index, mask):
    x = np.asarray(x, np.float32)
    qkv_w = np.asarray(qkv_w, np.float32)
    qkv_b = np.asarray(qkv_b, np.float32)
    proj_w = np.asarray(proj_w, np.float32)
    proj_b = np.asarray(proj_b, np.float32)
    rpe_table = np.asarray(rpe_table, np.float32)
    rpe_index = np.asarray(rpe_index)
    mask = np.asarray(mask)

    wqkv = qkv_w.T.copy()              # [C, 3C]
    wqkv[:, :C] *= SCALE               # fold q scaling
    wqkv = np.ascontiguousarray(wqkv).astype(BF16)
    qkb_full = qkv_b.copy()
    qkb_full[:C] *= SCALE
    qkb = np.ascontiguousarray(qkb_full[:2 * C].reshape(12, 128).T).astype(np.float32)
    wproj = np.ascontiguousarray(proj_w.T).astype(BF16)
    # v bias folded here: softmax rows sum to 1, so +vb before proj is exact
    pjb_full = proj_b + proj_w @ qkv_b[2 * C:]
    pjb = np.ascontiguousarray(pjb_full.reshape(6, 128).T).astype(np.float32)

    # relative position bias, transposed per head, exponentiated (applied
    # multiplicatively after exp): biasT[p, (h,mc,n)] = exp(bias[h,n,m])
    bias_hnm = rpe_table[rpe_index].reshape(N, N, H).transpose(2, 0, 1)  # [H,n,m]
    bT = np.zeros((H, 2, 128, N), np.float32)
    bT[:, 0, :, :] = bias_hnm.transpose(0, 2, 1)[:, 0:128, :]
    bT[:, 1, :68, :] = bias_hnm.transpose(0, 2, 1)[:, 128:196, :]
    biasT = np.ascontiguousarray(
        np.exp(bT.transpose(2, 0, 1, 3).reshape(128, H * 2 * N))).astype(BF16)

    in_maps = []
    for i in range(NCORES):
        xs = x[i * NB:(i + 1) * NB].reshape(TOK, C)
        xt = np.ascontiguousarray(xs.T).astype(BF16)
        mk = np.zeros((128, NB * 2), np.float32)
        msk = mask[i * NB:(i + 1) * NB]
        for b in range(NB):
            for mci, (mc, msz) in enumerate(MCH):
                col = np.where(msk[b, mc:mc + msz], 1.0, 0.0)
                mk[:msz, b * 2 + mci] = col
        in_maps.append({
            "xt": xt, "wqkv": wqkv, "qkb": qkb, "wproj": wproj, "pjb": pjb,
            "biasT": biasT, "maskp": np.ascontiguousarray(mk),
        })
    return in_maps


def _run(in_maps, trace=False, tmpdir=None):
    import sys, types
    # antenv.axon_hooks is absent on this image; rebuild the NTFF hook shim
    if trace and 'antenv.axon_hooks' not in sys.modules:
        try:
            import trn_agent_boot.trn_boot as tb
            hook = tb._ntff_profile_via_ctypes('/opt/axon/libaxon_pjrt.so')
            mod = types.ModuleType('antenv.axon_hooks')
            mod.get_axon_ntff_profile_hook = lambda: hook
            import antenv
            antenv.axon_hooks = mod
            sys.modules['antenv.axon_hooks'] = mod
            import concourse.bass_utils as bu
            bu.upload_artifacts = lambda d: d
        except Exception:
            trace = False
    from concourse.bass_utils import run_bass_kernel_spmd
    if 'nc' not in _cache:
        _cache['nc'] = _build_nc()
    return run_bass_kernel_spmd(_cache['nc'], in_maps, list(range(NCORES)),
                                trace=trace, tmpdir=tmpdir)


def kernel(x, qkv_w, qkv_b, proj_w, proj_b, rpe_table, rpe_index, mask):
    in_maps = _prep_in_maps(x, qkv_w, qkv_b, proj_w, proj_b, rpe_table,
                            rpe_index, mask)
    res = _run(in_maps, trace=False)
    out = np.empty((B, N, C), np.float32)
    for i in range(NCORES):
        oc = res.results[i]["out"]            # [C, TOK]
        out[i * NB:(i + 1) * NB] = oc.T.reshape(NB, N, C)
    return out
